# revision 19
# baseline (speedup 1.0000x reference)
"""Multi-Head Latent Attention on 8 Trainium2 NeuronCores.

Sharding: core c = (batch b = c//4) x (head-group g = c%4, 4 heads each).

Q path (no communication): the host fuses the query down- and
up-projections, Wq_eff = Wqd @ Wqu (and Wqr_eff = Wqd @ Wqr for rope),
so each core computes Q for its 4 heads directly from the full x of
its batch — an input every core already has. Only the KV latents go
through a collective: phase 1 computes kv_c for the core's 512-token
slice (token-sharded within the batch group) and one small AllGather
(0.5 MB/rank) over replica groups [[0..3],[4..7]] assembles the full
KV-latent tensor, fully hidden behind the Q-projection compute.

Each core then runs the K/V/K-rope up-projections for its heads over
all tokens, attention for its 4 heads, and a partial output
projection. Host sums the 4 partials per batch and adds the output
bias (plus the value-up bias folded through out_w, exact because
softmax rows sum to 1).

All on-device layouts are feature-major ("transposed"): x^T, kv_c^T,
K^T, Q^T, ctx^T, out^T — every matmul contraction lands on the
partition axis with zero transposes. Scores are computed as
scores^T[k, q] so probs^T feeds the context matmul directly; exp is
applied without max-subtraction (scores for this problem are in
[-1, 1], verified offline).

Rope: rot(y)[2i] = y[2i]cos_i - y[2i+1]sin_i, rot(y)[2i+1] =
y[2i]sin_i + y[2i+1]cos_i. We compute y = Wx + b once, produce the
pair-swapped copy with a partition-stride-2 SBUF->SBUF DMA, and fold
the sign pattern into the sin table (row 2i: -sin, row 2i+1: +sin),
so no second matmul set is needed.

Attention is software-pipelined over (head, key-block) units: the
score matmuls of unit i are emitted before the ctx matmul of unit
i-1 so the scalar-engine exp never stalls the PE. The softmax
denominator is accumulated on the Pool engine (probs tiles summed
into an f32 accumulator) with a single [P,1]-ones reduce matmul per
(head, q-chunk); the normalization chain is split in two parts
emitted 1 and 3 units after a head closes, hiding cross-engine
latency.

DMA queue assignment (to avoid head-of-line blocking):
  sync   (HWDGE): phase-1 x/Wd, fused Q weights, x-full tiles,
                  KV-latent reads (even), Wo loads, outT writes
  scalar (HWDGE): cos/sin, latent staging writes, KV-latent reads
                  (odd)
  gpsimd (SWDGE): small constants, K/V/Kr weights, rope swap copies,
                  collective trigger; Pool-engine ALU does the probs
                  accumulation
"""

import numpy as np
import ml_dtypes

import concourse.bass as bass
import concourse.mybir as mybir
from concourse.tile import TileContext
from concourse.bass_utils import run_bass_kernel_spmd

F32 = mybir.dt.float32
BF16 = mybir.dt.bfloat16
AF = mybir.ActivationFunctionType
BF = ml_dtypes.bfloat16

HIDDEN = 2048
NUM_HEADS = 16
HEAD_DIM = 128
KV_C = 512
Q_C = 1536
ROPE_DIM = 64
B, S = 2, 2048

P = 128
NH = 4          # heads per core
SC = 512        # free-dim chunk for projections / q-chunks
NKT = HIDDEN // P       # 16 k-tiles over the HIDDEN contraction
NKV = KV_C // P         # 4 kv-latent chunks
SCALE = float(1.0 / np.sqrt(HEAD_DIM + ROPE_DIM))
NEG = -1.0e5

RG = [[0, 1, 2, 3], [4, 5, 6, 7]]  # same-batch replica groups


def _split_waits(nc, maxw=1):
    """This container's walrus accepts at most one sem-wait per instruction;
    move excess waits onto same-engine NOPs inserted immediately before."""
    for fn in nc.m.functions:
        for bb in fn.blocks:
            newlist = []
            for ins in bb.instructions:
                si = ins.sync_info
                if si is not None and si.on_wait is not None and len(si.on_wait) > maxw:
                    waits = list(si.on_wait)
                    extra, keep = waits[:-maxw], waits[-maxw:]
                    for k, i in enumerate(range(0, len(extra), maxw)):
                        nop = mybir.InstNoOp(
                            name=f"{ins.name}-waitsplit-{k}", ins=[], outs=[]
                        )
                        nop.engine = ins.engine
                        nop.sync_info = mybir.SyncInfo(
                            on_wait=extra[i : i + maxw], on_update=[]
                        )
                        newlist.append(nop)
                    ins.sync_info = mybir.SyncInfo(
                        on_wait=keep, on_update=list(si.on_update or [])
                    )
                newlist.append(ins)
            bb.instructions = newlist


def build():
    nc = bass.Bass(num_devices=8)
    dt = nc.dram_tensor
    xTs = dt("xTs", [HIDDEN, SC], BF16, kind="ExternalInput")  # own slice
    xTf = dt("xTf", [HIDDEN, S], BF16, kind="ExternalInput")   # full batch
    Wd = dt("Wd", [HIDDEN, KV_C], BF16, kind="ExternalInput")
    bd = dt("bd", [P, NKV], F32, kind="ExternalInput")
    Wku = dt("Wku", [KV_C, NH * HEAD_DIM], BF16, kind="ExternalInput")
    bku = dt("bku", [P, 4], F32, kind="ExternalInput")
    Wvu = dt("Wvu", [KV_C, NH * HEAD_DIM], BF16, kind="ExternalInput")
    Wkr = dt("Wkr", [KV_C, NH * ROPE_DIM], BF16, kind="ExternalInput")
    bkr = dt("bkr", [P, 2], F32, kind="ExternalInput")
    Wqf = dt("Wqf", [HIDDEN, NH * HEAD_DIM], BF16, kind="ExternalInput")
    bqf = dt("bqf", [P, 4], F32, kind="ExternalInput")
    Wqrf = dt("Wqrf", [HIDDEN, NH * ROPE_DIM], BF16, kind="ExternalInput")
    bqrf = dt("bqrf", [P, 2], F32, kind="ExternalInput")
    Wo = dt("Wo", [NH * HEAD_DIM, HIDDEN], BF16, kind="ExternalInput")
    cos2 = dt("cos2", [P, S], BF16, kind="ExternalInput")
    sina = dt("sina", [P, S], BF16, kind="ExternalInput")
    tri = dt("tri", [P, P], F32, kind="ExternalInput")
    outT = dt("outT", [HIDDEN, S], BF16, kind="ExternalOutput")

    NSC = S // SC  # 4 token chunks

    with TileContext(nc) as tc:
        with (
            tc.tile_pool(name="const", bufs=1) as pc,
            tc.tile_pool(name="dram", bufs=1, space="DRAM") as pdram,
            tc.tile_pool(name="qkv", bufs=1) as pq,
            tc.tile_pool(name="w2", bufs=1) as pw2,
        ):
            # --- constants ---
            cos_sb = pc.tile([P, S], BF16)
            sin_sb = pc.tile([P, S], BF16)
            nc.scalar.dma_start(cos_sb[:], cos2[:])
            nc.scalar.dma_start(sin_sb[:], sina[:])
            tri_sb = pc.tile([P, P], F32)
            nc.gpsimd.dma_start(tri_sb[:], tri[:])
            bd_sb = pc.tile([P, NKV], F32)
            nc.gpsimd.dma_start(bd_sb[:], bd[:])
            bku_sb = pc.tile([P, 4], F32)
            nc.gpsimd.dma_start(bku_sb[:], bku[:])
            bkr_sb = pc.tile([P, 2], F32)
            nc.gpsimd.dma_start(bkr_sb[:], bkr[:])
            bqf_sb = pc.tile([P, 4], F32)
            nc.gpsimd.dma_start(bqf_sb[:], bqf[:])
            bqrf_sb = pc.tile([P, 2], F32)
            nc.gpsimd.dma_start(bqrf_sb[:], bqrf[:])
            ones_row = pc.tile([1, P], BF16)
            nc.vector.memset(ones_row[:], 1.0)
            ones_col = pc.tile([P, 1], BF16)
            nc.vector.memset(ones_col[:], 1.0)

            # collective bounce buffers (DRAM)
            cc1_in = pdram.tile([P, NKV, SC], BF16)
            cc1_out = pdram.tile([4, P, NKV, SC], BF16)

            # phase-2/3 operands (live until the end)
            kc_sb = pq.tile([P, NH, S], BF16)
            kr_sb = pq.tile([P, 2, S], BF16)
            qc_sb = pq.tile([P, NH, S], BF16)
            qr_sb = pq.tile([P, 2, S], BF16)
            v_sb = pq.tile([P, S // P, NH * HEAD_DIM], BF16)

            # weights: fused Q on scalar queue (needed early), K/V/Kr gpsimd
            wqf_t = pw2.tile([P, NKT, NH * HEAD_DIM], BF16)
            nc.scalar.dma_start(
                wqf_t[:], Wqf.rearrange("(t p) m -> p t m", p=P)
            )
            wqrf_t = pw2.tile([P, NKT, NH * ROPE_DIM], BF16)
            nc.scalar.dma_start(
                wqrf_t[:], Wqrf.rearrange("(t p) m -> p t m", p=P)
            )
            wku_t = pw2.tile([P, NKV, NH * HEAD_DIM], BF16)
            wvu_t = pw2.tile([P, NKV, NH * HEAD_DIM], BF16)
            wkr_t = pw2.tile([P, NKV, NH * ROPE_DIM], BF16)

            # ------- phase 1: KV-latent down projection, OWN slice -------
            with (
                tc.tile_pool(name="p1", bufs=1) as p1,
                tc.tile_pool(name="p1w", bufs=2) as p1w,
                tc.tile_pool(name="p1l", bufs=4) as p1l,
                tc.tile_pool(name="ps1", bufs=4, space="PSUM") as ps1,
            ):
                xTr = xTs.rearrange("(t p) s -> p t s", p=P)
                wd_first = p1w.tile([P, NKT, P], BF16, tag="wd")
                wdr0 = Wd[:, 0:P].rearrange("(t p) m -> p t m", p=P)
                for qtr in range(4):
                    nc.sync.dma_start(
                        wd_first[:, 4 * qtr : 4 * qtr + 4, :],
                        wdr0[:, 4 * qtr : 4 * qtr + 4, :],
                    )
                xt_tiles = []
                for k in range(NKT):
                    t = p1.tile([P, SC], BF16, tag=f"xt{k}")
                    nc.sync.dma_start(t[:], xTr[:, k, :])
                    xt_tiles.append(t)
                for m in range(NKV):
                    if m == 0:
                        wd_t = wd_first
                    else:
                        wd_t = p1w.tile([P, NKT, P], BF16, tag="wd")
                        nc.sync.dma_start(
                            wd_t[:],
                            Wd[:, m * P : (m + 1) * P].rearrange(
                                "(t p) m -> p t m", p=P
                            ),
                        )
                    ps = ps1.tile([P, SC], F32, tag="mm")
                    for k in range(NKT):
                        nc.tensor.matmul(
                            ps[:],
                            wd_t[:, k, :],
                            xt_tiles[k][:],
                            start=(k == 0),
                            stop=(k == NKT - 1),
                        )
                    lat = p1l.tile([P, SC], BF16, tag="lat")
                    nc.vector.tensor_scalar_add(
                        lat[:], ps[:], bd_sb[:, m : m + 1]
                    )
                    nc.scalar.dma_start(cc1_in[:, m, :], lat[:])
                nc.gpsimd.collective_compute(
                    "AllGather", mybir.AluOpType.bypass,
                    replica_groups=RG,
                    ins=[cc1_in[:].opt()],
                    outs=[cc1_out[:].opt()],
                )
                # K/V/Kr weights on the gpsimd queue
                nc.gpsimd.dma_start(
                    wku_t[:], Wku.rearrange("(t p) m -> p t m", p=P)
                )
                nc.gpsimd.dma_start(
                    wvu_t[:], Wvu.rearrange("(t p) m -> p t m", p=P)
                )
                nc.gpsimd.dma_start(
                    wkr_t[:], Wkr.rearrange("(t p) m -> p t m", p=P)
                )

            # ---- phase 2a: fused Q projection from full x (no comm) ----
            with (
                tc.tile_pool(name="pxf", bufs=2) as pxf,
                tc.tile_pool(name="p2t", bufs=3) as p2t,
                tc.tile_pool(name="ps2", bufs=4, space="PSUM") as ps2,
            ):
                def rope_finish(dst, psA, bias, sl):
                    """dst = (psA+bias)*cos + swap(psA+bias)*sin_alt"""
                    tA = p2t.tile([P, SC], F32, tag="ropeA", name="tA")
                    nc.vector.tensor_scalar_add(tA[:], psA[:], bias)
                    sw = p2t.tile([P, SC], F32, tag="ropeS", name="sw")
                    nc.gpsimd.dma_start(sw[0::2, :], tA[1::2, :])
                    nc.gpsimd.dma_start(sw[1::2, :], tA[0::2, :])
                    tC = p2t.tile([P, SC], F32, tag="ropeC", name="tC")
                    nc.vector.tensor_tensor(
                        tC[:], tA[:], cos_sb[:, sl], mybir.AluOpType.mult
                    )
                    nc.vector.tensor_tensor(
                        sw[:], sw[:], sin_sb[:, sl], mybir.AluOpType.mult
                    )
                    nc.vector.tensor_tensor(
                        dst, tC[:], sw[:], mybir.AluOpType.add
                    )

                xfr = xTf.rearrange("(t p) s -> p t s", p=P)
                for g in range(NSC):
                    sl = slice(g * SC, (g + 1) * SC)
                    xf_tiles = []
                    for k in range(NKT):
                        t = pxf.tile([P, SC], BF16, tag=f"xf{k}")
                        nc.sync.dma_start(t[:], xfr[:, k, sl])
                        xf_tiles.append(t)
                    for m in range(NH):
                        ps = ps2.tile([P, SC], F32, tag="mm")
                        for k in range(NKT):
                            nc.tensor.matmul(
                                ps[:],
                                wqf_t[:, k, m * P : (m + 1) * P],
                                xf_tiles[k][:],
                                start=(k == 0),
                                stop=(k == NKT - 1),
                            )
                        nc.vector.tensor_scalar_add(
                            qc_sb[:, m, sl], ps[:], bqf_sb[:, m : m + 1]
                        )
                    for m in range(2):
                        psA = ps2.tile([P, SC], F32, tag="mm")
                        for k in range(NKT):
                            nc.tensor.matmul(
                                psA[:],
                                wqrf_t[:, k, m * P : (m + 1) * P],
                                xf_tiles[k][:],
                                start=(k == 0),
                                stop=(k == NKT - 1),
                            )
                        rope_finish(
                            qr_sb[:, m, sl], psA, bqrf_sb[:, m : m + 1], sl
                        )

                # ---- phase 2b: K/V/K-rope from gathered KV latents ----
                with tc.tile_pool(name="lkv", bufs=2) as plkv:
                    for g in range(NSC):
                        sl = slice(g * SC, (g + 1) * SC)
                        lkv = plkv.tile([P, NKV, SC], BF16, tag="kv")
                        keng = nc.sync if g % 2 == 0 else nc.scalar
                        keng.dma_start(lkv[:], cc1_out[g])
                        for m in range(NH):
                            ps = ps2.tile([P, SC], F32, tag="mm")
                            for k in range(NKV):
                                nc.tensor.matmul(
                                    ps[:],
                                    wku_t[:, k, m * P : (m + 1) * P],
                                    lkv[:, k, :],
                                    start=(k == 0),
                                    stop=(k == NKV - 1),
                                )
                            nc.vector.tensor_scalar_add(
                                kc_sb[:, m, sl], ps[:], bku_sb[:, m : m + 1]
                            )
                        for t in range(4 * g, 4 * g + 4):
                            ps = ps2.tile([P, NH * HEAD_DIM], F32, tag="mm")
                            for k in range(NKV):
                                nc.tensor.matmul(
                                    ps[:],
                                    lkv[:, k, (t - 4 * g) * P : (t - 4 * g + 1) * P],
                                    wvu_t[:, k, :],
                                    start=(k == 0),
                                    stop=(k == NKV - 1),
                                )
                            nc.vector.tensor_copy(v_sb[:, t, :], ps[:])
                        for m in range(2):
                            psA = ps2.tile([P, SC], F32, tag="mm")
                            for k in range(NKV):
                                nc.tensor.matmul(
                                    psA[:],
                                    wkr_t[:, k, m * P : (m + 1) * P],
                                    lkv[:, k, :],
                                    start=(k == 0), stop=(k == NKV - 1),
                                )
                            rope_finish(
                                kr_sb[:, m, sl], psA, bkr_sb[:, m : m + 1], sl
                            )

            # ---------- phase 3: attention + inline out-proj ----------
            with (
                tc.tile_pool(name="at", bufs=12) as pat,
                tc.tile_pool(name="atx", bufs=2) as patx,
                tc.tile_pool(name="att", bufs=2) as patt,
                tc.tile_pool(name="acc", bufs=2) as pacc,
                tc.tile_pool(name="out", bufs=3) as pout,
                tc.tile_pool(name="ow", bufs=3) as pow_,
                tc.tile_pool(name="ps_sc", bufs=3, space="PSUM") as ps_sc,
                tc.tile_pool(name="ps_acc", bufs=2, space="PSUM") as ps_acc,
                tc.tile_pool(name="ps_red", bufs=1, space="PSUM") as ps_red,
                tc.tile_pool(name="ps_m", bufs=2, space="PSUM") as ps_m,
            ):
                for qc in range(NSC):
                    nkb = 4 * qc + 4
                    ctx_q = patx.tile([P, NH, SC], BF16, tag="ctx")
                    acc = {}
                    sacc = {}
                    nstate = {}

                    def emit_scores(h, kb):
                        hc = h // 2
                        hp = (h % 2) * ROPE_DIM
                        ksl = slice(kb * P, (kb + 1) * P)
                        diag = kb >= 4 * qc
                        c = (kb - 4 * qc) * P if diag else 0
                        qs0 = qc * SC + c
                        ps = ps_sc.tile([P, SC], F32, tag="sc", name="ps")
                        nc.tensor.matmul(
                            ps[:, c:],
                            kc_sb[:, h, ksl],
                            qc_sb[:, h, qs0 : (qc + 1) * SC],
                            start=True, stop=False,
                        )
                        nc.tensor.matmul(
                            ps[:, c:],
                            kr_sb[hp : hp + ROPE_DIM, hc, ksl],
                            qr_sb[hp : hp + ROPE_DIM, hc,
                                  qs0 : (qc + 1) * SC],
                            start=False, stop=True,
                        )
                        probs = pat.tile([P, SC], BF16, tag="probs",
                                         name="probs")
                        if diag:
                            nc.vector.tensor_tensor(
                                ps[:, c : c + P],
                                ps[:, c : c + P],
                                tri_sb[:],
                                mybir.AluOpType.add,
                            )
                        nc.scalar.activation(
                            probs[:, c:], ps[:, c:], AF.Exp, scale=SCALE,
                        )
                        return (h, kb, probs, c)

                    def emit_ctx(unit):
                        h, kb, probs, c = unit
                        nc.tensor.matmul(
                            acc[h][:, c:],
                            v_sb[:, kb, h * P : (h + 1) * P],
                            probs[:, c:],
                            start=(kb == 0), stop=(kb == nkb - 1),
                        )
                        if kb == 0:
                            nc.gpsimd.tensor_copy(sacc[h][:], probs[:])
                        elif kb % 2 == 1:
                            nc.vector.tensor_tensor(
                                sacc[h][:, c:], sacc[h][:, c:],
                                probs[:, c:], mybir.AluOpType.add,
                            )
                        else:
                            nc.gpsimd.tensor_tensor(
                                sacc[h][:, c:], sacc[h][:, c:],
                                probs[:, c:], mybir.AluOpType.add,
                            )
                        return h if kb == nkb - 1 else None

                    def emit_norm_a(h):
                        acc16 = patt.tile([P, SC], BF16, tag="acc16",
                                          name="acc16")
                        nc.gpsimd.tensor_copy(acc16[:], sacc[h][:])
                        red = ps_red.tile([1, SC], F32, tag="red", name="red")
                        nc.tensor.matmul(
                            red[:], ones_col[:], acc16[:],
                            start=True, stop=True,
                        )
                        rf = patt.tile([1, SC], F32, tag="recip", name="rf")
                        nc.vector.reciprocal(rf[:], red[0:1, :])
                        r16 = patt.tile([1, SC], BF16, tag="r16", name="r16")
                        nc.vector.tensor_copy(r16[:], rf[:])
                        nstate[h] = r16

                    def emit_norm_b(h):
                        r16 = nstate.pop(h)
                        psb = ps_m.tile([P, SC], F32, tag="m", name="psb")
                        nc.tensor.matmul(
                            psb[:], ones_row[:], r16[:],
                            start=True, stop=True,
                        )
                        rbc = patt.tile([P, SC], BF16, tag="rbc", name="rbc")
                        nc.scalar.copy(rbc[:], psb[:])
                        nc.vector.tensor_tensor(
                            ctx_q[:, h, :], acc[h][:], rbc[:],
                            mybir.AluOpType.mult,
                        )

                    units = [(h, kb) for h in range(NH) for kb in range(nkb)]
                    n = len(units)
                    state = {}
                    sched = {}
                    for i in range(n + 4):
                        if i < n:
                            h, kb = units[i]
                            if kb == 0:
                                acc[h] = ps_acc.tile([P, SC], F32, tag="ctx",
                                                     name="pctx")
                                sacc[h] = pacc.tile([P, SC], F32, tag="sacc",
                                                    name="sacc")
                            state[i] = emit_scores(h, kb)
                        if 0 <= i - 1 < n:
                            h_closed = emit_ctx(state.pop(i - 1))
                            if h_closed is not None:
                                sched.setdefault(i + 1, []).append(
                                    ("a", h_closed))
                                sched.setdefault(i + 3, []).append(
                                    ("b", h_closed))
                        for kind, hh in sched.pop(i, []):
                            (emit_norm_a if kind == "a" else emit_norm_b)(hh)

                    # out-projection for this q-chunk
                    for m in range(NKT):
                        wo_t = pow_.tile([P, NH, P], BF16, tag="wo")
                        nc.sync.dma_start(
                            wo_t[:],
                            Wo[:, m * P : (m + 1) * P].rearrange(
                                "(t p) m -> p t m", p=P
                            ),
                        )
                        ps = ps_m.tile([P, SC], F32, tag="m", name="ps")
                        for k in range(NH):
                            nc.tensor.matmul(
                                ps[:],
                                wo_t[:, k, :],
                                ctx_q[:, k, :],
                                start=(k == 0),
                                stop=(k == NH - 1),
                            )
                        og = pout.tile([P, SC], BF16, tag="og")
                        nc.scalar.copy(og[:], ps[:])
                        nc.scalar.dma_start(
                            outT[m * P : (m + 1) * P,
                                 qc * SC : (qc + 1) * SC],
                            og[:],
                        )
    _split_waits(nc)
    return nc


def _col_bias(b, nm):
    """[nm*128] -> [128, nm] (column m = bias for feature chunk m)."""
    return np.ascontiguousarray(b.reshape(nm, P).T).astype(np.float32)


_NC = None


def kernel(**inputs):
    global _NC
    inp = {k: np.asarray(v) for k, v in inputs.items()}
    x = inp["x"].astype(np.float32)

    pos = np.arange(S, dtype=np.float64)
    inv = 1.0 / (10000.0 ** (np.arange(0, ROPE_DIM, 2, np.float64) / ROPE_DIM))
    ang = pos[None, :] * inv[:, None]          # [32, S]
    idx = (np.arange(P) % ROPE_DIM) // 2       # row -> freq index
    cos2 = np.cos(ang)[idx].astype(BF)
    sgn = np.where(np.arange(P) % 2 == 0, -1.0, 1.0)[:, None]
    sina = (np.sin(ang)[idx] * sgn).astype(BF)
    tri = np.where(
        np.arange(P)[None, :] >= np.arange(P)[:, None], 0.0, NEG
    ).astype(np.float32)

    qdw = inp["query_down_w"].astype(np.float32)
    qdb = inp["query_down_b"].astype(np.float32)
    quw = inp["query_up_w"].astype(np.float32)
    qrw = inp["query_rope_w"].astype(np.float32)

    in_maps = []
    for c in range(8):
        b, g = c // 4, c % 4
        h0 = g * NH
        csl = slice(h0 * HEAD_DIM, (h0 + NH) * HEAD_DIM)
        rsl = slice(h0 * ROPE_DIM, (h0 + NH) * ROPE_DIM)
        wqf = qdw @ quw[:, csl]
        bqf = qdb @ quw[:, csl] + inp["query_up_b"][csl].astype(np.float32)
        wqrf = qdw @ qrw[:, rsl]
        bqrf = qdb @ qrw[:, rsl] + inp["query_rope_b"][rsl].astype(np.float32)
        xt = np.ascontiguousarray(x[b].T).astype(BF)
        in_maps.append(
            {
                "xTs": np.ascontiguousarray(xt[:, g * SC : (g + 1) * SC]),
                "xTf": xt,
                "Wd": inp["kv_down_w"].astype(BF),
                "bd": _col_bias(inp["kv_down_b"], NKV),
                "Wku": inp["key_up_w"][:, csl].astype(BF),
                "bku": _col_bias(inp["key_up_b"][csl], 4),
                "Wvu": inp["value_up_w"][:, csl].astype(BF),
                "Wkr": inp["key_rope_w"][:, rsl].astype(BF),
                "bkr": _col_bias(inp["key_rope_b"][rsl].astype(np.float32), 2),
                "Wqf": wqf.astype(BF),
                "bqf": _col_bias(bqf, 4),
                "Wqrf": wqrf.astype(BF),
                "bqrf": _col_bias(bqrf, 2),
                "Wo": inp["out_w"][csl, :].astype(BF),
                "cos2": cos2,
                "sina": sina,
                "tri": tri,
            }
        )

    if _NC is None:
        _NC = build()
    res = run_bass_kernel_spmd(_NC, in_maps, core_ids=list(range(8)))

    corr = (
        inp["value_up_b"].astype(np.float32) @ inp["out_w"].astype(np.float32)
        + inp["out_b"].astype(np.float32)
    )
    out = np.empty((B, S, HIDDEN), np.float32)
    for b in range(B):
        acc = res.results[b * 4]["outT"].astype(np.float32)
        for g in range(1, 4):
            acc += res.results[b * 4 + g]["outT"].astype(np.float32)
        out[b] = acc.T + corr[None, :]
    return out


# revision 20
# speedup vs baseline: 1.0587x; 1.0587x over previous
"""Multi-Head Latent Attention on 8 Trainium2 NeuronCores.

Sharding: core c = (batch b = c//4) x (head-group g = c%4, 4 heads each).

Q path (no communication): the host fuses the query down- and
up-projections, Wq_eff = Wqd @ Wqu (and Wqr_eff = Wqd @ Wqr for rope),
so each core computes Q for its 4 heads directly from the full x of
its batch — an input every core already has. Only the KV latents go
through a collective: phase 1 computes kv_c for the core's 512-token
slice (token-sharded within the batch group) and one small AllGather
(0.5 MB/rank) over replica groups [[0..3],[4..7]] assembles the full
KV-latent tensor, fully hidden behind the Q-projection compute.

Each core then runs the K/V/K-rope up-projections for its heads over
all tokens, attention for its 4 heads, and a partial output
projection. Host sums the 4 partials per batch and adds the output
bias (plus the value-up bias folded through out_w, exact because
softmax rows sum to 1).

All on-device layouts are feature-major ("transposed"): x^T, kv_c^T,
K^T, Q^T, ctx^T, out^T — every matmul contraction lands on the
partition axis with zero transposes. Scores are computed as
scores^T[k, q] so probs^T feeds the context matmul directly; exp is
applied without max-subtraction (scores for this problem are in
[-1, 1], verified offline).

Rope: rot(y)[2i] = y[2i]cos_i - y[2i+1]sin_i, rot(y)[2i+1] =
y[2i]sin_i + y[2i+1]cos_i. We compute y = Wx + b once, produce the
pair-swapped copy with a partition-stride-2 SBUF->SBUF DMA, and fold
the sign pattern into the sin table (row 2i: -sin, row 2i+1: +sin),
so no second matmul set is needed.

Attention is software-pipelined over (head, key-block) units: the
score matmuls of unit i are emitted before the ctx matmul of unit
i-1 so the scalar-engine exp never stalls the PE. The softmax
denominator is accumulated on the Pool engine (probs tiles summed
into an f32 accumulator) with a single [P,1]-ones reduce matmul per
(head, q-chunk); the normalization chain is split in two parts
emitted 1 and 3 units after a head closes, hiding cross-engine
latency.

DMA queue assignment (to avoid head-of-line blocking):
  sync   (HWDGE): phase-1 x/Wd, fused Q weights, x-full tiles,
                  KV-latent reads (even), Wo loads, outT writes
  scalar (HWDGE): cos/sin, latent staging writes, KV-latent reads
                  (odd)
  gpsimd (SWDGE): small constants, K/V/Kr weights, rope swap copies,
                  collective trigger; Pool-engine ALU does the probs
                  accumulation
"""

import numpy as np
import ml_dtypes

import concourse.bass as bass
import concourse.mybir as mybir
from concourse.tile import TileContext
from concourse.bass_utils import run_bass_kernel_spmd

F32 = mybir.dt.float32
BF16 = mybir.dt.bfloat16
AF = mybir.ActivationFunctionType
BF = ml_dtypes.bfloat16

HIDDEN = 2048
NUM_HEADS = 16
HEAD_DIM = 128
KV_C = 512
Q_C = 1536
ROPE_DIM = 64
B, S = 2, 2048

P = 128
NH = 4          # heads per core
SC = 512        # free-dim chunk for projections / q-chunks
NKT = HIDDEN // P       # 16 k-tiles over the HIDDEN contraction
NKV = KV_C // P         # 4 kv-latent chunks
SCALE = float(1.0 / np.sqrt(HEAD_DIM + ROPE_DIM))
NEG = -1.0e5

RG = [[0, 1, 2, 3], [4, 5, 6, 7]]  # same-batch replica groups


def _split_waits(nc, maxw=1):
    """This container's walrus accepts at most one sem-wait per instruction;
    move excess waits onto same-engine NOPs inserted immediately before."""
    for fn in nc.m.functions:
        for bb in fn.blocks:
            newlist = []
            for ins in bb.instructions:
                si = ins.sync_info
                if si is not None and si.on_wait is not None and len(si.on_wait) > maxw:
                    waits = list(si.on_wait)
                    extra, keep = waits[:-maxw], waits[-maxw:]
                    for k, i in enumerate(range(0, len(extra), maxw)):
                        nop = mybir.InstNoOp(
                            name=f"{ins.name}-waitsplit-{k}", ins=[], outs=[]
                        )
                        nop.engine = ins.engine
                        nop.sync_info = mybir.SyncInfo(
                            on_wait=extra[i : i + maxw], on_update=[]
                        )
                        newlist.append(nop)
                    ins.sync_info = mybir.SyncInfo(
                        on_wait=keep, on_update=list(si.on_update or [])
                    )
                newlist.append(ins)
            bb.instructions = newlist


def build():
    nc = bass.Bass(num_devices=8)
    dt = nc.dram_tensor
    xTs = dt("xTs", [HIDDEN, SC], BF16, kind="ExternalInput")  # own slice
    xTf = dt("xTf", [HIDDEN, S], BF16, kind="ExternalInput")   # full batch
    Wd = dt("Wd", [HIDDEN, KV_C], BF16, kind="ExternalInput")
    bd = dt("bd", [P, NKV], F32, kind="ExternalInput")
    Wku = dt("Wku", [KV_C, NH * HEAD_DIM], BF16, kind="ExternalInput")
    bku = dt("bku", [P, 4], F32, kind="ExternalInput")
    Wvu = dt("Wvu", [KV_C, NH * HEAD_DIM], BF16, kind="ExternalInput")
    Wkr = dt("Wkr", [KV_C, NH * ROPE_DIM], BF16, kind="ExternalInput")
    bkr = dt("bkr", [P, 2], F32, kind="ExternalInput")
    Wqf = dt("Wqf", [HIDDEN, NH * HEAD_DIM], BF16, kind="ExternalInput")
    bqf = dt("bqf", [P, 4], F32, kind="ExternalInput")
    Wqrf = dt("Wqrf", [HIDDEN, NH * ROPE_DIM], BF16, kind="ExternalInput")
    bqrf = dt("bqrf", [P, 2], F32, kind="ExternalInput")
    Wo = dt("Wo", [NH * HEAD_DIM, HIDDEN], BF16, kind="ExternalInput")
    cos2 = dt("cos2", [P, S], BF16, kind="ExternalInput")
    sina = dt("sina", [P, S], BF16, kind="ExternalInput")
    tri = dt("tri", [P, P], F32, kind="ExternalInput")
    outT = dt("outT", [HIDDEN, S], BF16, kind="ExternalOutput")

    NSC = S // SC  # 4 token chunks

    with TileContext(nc) as tc:
        with (
            tc.tile_pool(name="const", bufs=1) as pc,
            tc.tile_pool(name="dram", bufs=1, space="DRAM") as pdram,
            tc.tile_pool(name="qkv", bufs=1) as pq,
            tc.tile_pool(name="w2", bufs=1) as pw2,
        ):
            # --- constants ---
            cos_sb = pc.tile([P, S], BF16)
            sin_sb = pc.tile([P, S], BF16)
            nc.scalar.dma_start(cos_sb[:], cos2[:])
            nc.scalar.dma_start(sin_sb[:], sina[:])
            tri_sb = pc.tile([P, P], F32)
            nc.gpsimd.dma_start(tri_sb[:], tri[:])
            bd_sb = pc.tile([P, NKV], F32)
            nc.gpsimd.dma_start(bd_sb[:], bd[:])
            bku_sb = pc.tile([P, 4], F32)
            nc.gpsimd.dma_start(bku_sb[:], bku[:])
            bkr_sb = pc.tile([P, 2], F32)
            nc.gpsimd.dma_start(bkr_sb[:], bkr[:])
            bqf_sb = pc.tile([P, 4], F32)
            nc.gpsimd.dma_start(bqf_sb[:], bqf[:])
            bqrf_sb = pc.tile([P, 2], F32)
            nc.gpsimd.dma_start(bqrf_sb[:], bqrf[:])
            ones_row = pc.tile([1, P], BF16)
            nc.vector.memset(ones_row[:], 1.0)
            ones_col = pc.tile([P, 1], BF16)
            nc.vector.memset(ones_col[:], 1.0)

            # collective bounce buffers (DRAM)
            cc1_in = pdram.tile([P, NKV, SC], BF16)
            cc1_out = pdram.tile([4, P, NKV, SC], BF16)

            # phase-2/3 operands (live until the end)
            kc_sb = pq.tile([P, NH, S], BF16)
            kr_sb = pq.tile([P, 2, S], BF16)
            qc_sb = pq.tile([P, NH, S], BF16)
            qr_sb = pq.tile([P, 2, S], BF16)
            v_sb = pq.tile([P, S // P, NH * HEAD_DIM], BF16)

            # weights: fused Q on scalar queue (needed early), K/V/Kr gpsimd
            wqf_t = pw2.tile([P, NKT, NH * HEAD_DIM], BF16)
            nc.scalar.dma_start(
                wqf_t[:], Wqf.rearrange("(t p) m -> p t m", p=P)
            )
            wqrf_t = pw2.tile([P, NKT, NH * ROPE_DIM], BF16)
            nc.scalar.dma_start(
                wqrf_t[:], Wqrf.rearrange("(t p) m -> p t m", p=P)
            )
            wku_t = pw2.tile([P, NKV, NH * HEAD_DIM], BF16)
            wvu_t = pw2.tile([P, NKV, NH * HEAD_DIM], BF16)
            wkr_t = pw2.tile([P, NKV, NH * ROPE_DIM], BF16)

            # ------- phase 1: KV-latent down projection, OWN slice -------
            with (
                tc.tile_pool(name="p1", bufs=1) as p1,
                tc.tile_pool(name="p1w", bufs=2) as p1w,
                tc.tile_pool(name="p1l", bufs=4) as p1l,
                tc.tile_pool(name="ps1", bufs=4, space="PSUM") as ps1,
            ):
                xTr = xTs.rearrange("(t p) s -> p t s", p=P)
                wd_first = p1w.tile([P, NKT, P], BF16, tag="wd")
                wdr0 = Wd[:, 0:P].rearrange("(t p) m -> p t m", p=P)
                for qtr in range(4):
                    nc.sync.dma_start(
                        wd_first[:, 4 * qtr : 4 * qtr + 4, :],
                        wdr0[:, 4 * qtr : 4 * qtr + 4, :],
                    )
                xt_tiles = []
                for k in range(NKT):
                    t = p1.tile([P, SC], BF16, tag=f"xt{k}")
                    nc.sync.dma_start(t[:], xTr[:, k, :])
                    xt_tiles.append(t)
                for m in range(NKV):
                    if m == 0:
                        wd_t = wd_first
                    else:
                        wd_t = p1w.tile([P, NKT, P], BF16, tag="wd")
                        nc.sync.dma_start(
                            wd_t[:],
                            Wd[:, m * P : (m + 1) * P].rearrange(
                                "(t p) m -> p t m", p=P
                            ),
                        )
                    ps = ps1.tile([P, SC], F32, tag="mm")
                    for k in range(NKT):
                        nc.tensor.matmul(
                            ps[:],
                            wd_t[:, k, :],
                            xt_tiles[k][:],
                            start=(k == 0),
                            stop=(k == NKT - 1),
                        )
                    lat = p1l.tile([P, SC], BF16, tag="lat")
                    nc.vector.tensor_scalar_add(
                        lat[:], ps[:], bd_sb[:, m : m + 1]
                    )
                    nc.scalar.dma_start(cc1_in[:, m, :], lat[:])
                nc.gpsimd.collective_compute(
                    "AllGather", mybir.AluOpType.bypass,
                    replica_groups=RG,
                    ins=[cc1_in[:].opt()],
                    outs=[cc1_out[:].opt()],
                )
                # K/V/Kr weights on the gpsimd queue
                nc.gpsimd.dma_start(
                    wku_t[:], Wku.rearrange("(t p) m -> p t m", p=P)
                )
                nc.gpsimd.dma_start(
                    wvu_t[:], Wvu.rearrange("(t p) m -> p t m", p=P)
                )
                nc.gpsimd.dma_start(
                    wkr_t[:], Wkr.rearrange("(t p) m -> p t m", p=P)
                )

            # ---- phase 2a: fused Q projection from full x (no comm) ----
            with (
                tc.tile_pool(name="pxf", bufs=2) as pxf,
                tc.tile_pool(name="p2t", bufs=3) as p2t,
                tc.tile_pool(name="ps2", bufs=4, space="PSUM") as ps2,
            ):
                def rope_finish(dst, psA, bias, sl):
                    """dst = (psA+bias)*cos + swap(psA+bias)*sin_alt"""
                    tA = p2t.tile([P, SC], F32, tag="ropeA", name="tA")
                    nc.vector.tensor_scalar_add(tA[:], psA[:], bias)
                    sw = p2t.tile([P, SC], F32, tag="ropeS", name="sw")
                    nc.gpsimd.dma_start(sw[0::2, :], tA[1::2, :])
                    nc.gpsimd.dma_start(sw[1::2, :], tA[0::2, :])
                    tC = p2t.tile([P, SC], F32, tag="ropeC", name="tC")
                    nc.vector.tensor_tensor(
                        tC[:], tA[:], cos_sb[:, sl], mybir.AluOpType.mult
                    )
                    nc.vector.tensor_tensor(
                        sw[:], sw[:], sin_sb[:, sl], mybir.AluOpType.mult
                    )
                    nc.vector.tensor_tensor(
                        dst, tC[:], sw[:], mybir.AluOpType.add
                    )

                xfr = xTf.rearrange("(t p) s -> p t s", p=P)
                for g in range(NSC):
                    sl = slice(g * SC, (g + 1) * SC)
                    xf_tiles = []
                    for k in range(NKT):
                        t = pxf.tile([P, SC], BF16, tag=f"xf{k}")
                        nc.sync.dma_start(t[:], xfr[:, k, sl])
                        xf_tiles.append(t)
                    for m in range(NH):
                        ps = ps2.tile([P, SC], F32, tag="mm")
                        for k in range(NKT):
                            nc.tensor.matmul(
                                ps[:],
                                wqf_t[:, k, m * P : (m + 1) * P],
                                xf_tiles[k][:],
                                start=(k == 0),
                                stop=(k == NKT - 1),
                            )
                        nc.vector.tensor_scalar_add(
                            qc_sb[:, m, sl], ps[:], bqf_sb[:, m : m + 1]
                        )
                    for m in range(2):
                        psA = ps2.tile([P, SC], F32, tag="mm")
                        for k in range(NKT):
                            nc.tensor.matmul(
                                psA[:],
                                wqrf_t[:, k, m * P : (m + 1) * P],
                                xf_tiles[k][:],
                                start=(k == 0),
                                stop=(k == NKT - 1),
                            )
                        rope_finish(
                            qr_sb[:, m, sl], psA, bqrf_sb[:, m : m + 1], sl
                        )

                # ---- phase 2b: K/V/K-rope from gathered KV latents ----
                with tc.tile_pool(name="lkv", bufs=2) as plkv:
                    for g in range(NSC):
                        sl = slice(g * SC, (g + 1) * SC)
                        lkv = plkv.tile([P, NKV, SC], BF16, tag="kv")
                        keng = nc.sync if g % 2 == 0 else nc.scalar
                        keng.dma_start(lkv[:], cc1_out[g])
                        for m in range(NH):
                            ps = ps2.tile([P, SC], F32, tag="mm")
                            for k in range(NKV):
                                nc.tensor.matmul(
                                    ps[:],
                                    wku_t[:, k, m * P : (m + 1) * P],
                                    lkv[:, k, :],
                                    start=(k == 0),
                                    stop=(k == NKV - 1),
                                )
                            nc.vector.tensor_scalar_add(
                                kc_sb[:, m, sl], ps[:], bku_sb[:, m : m + 1]
                            )
                        for t in range(4 * g, 4 * g + 4):
                            ps = ps2.tile([P, NH * HEAD_DIM], F32, tag="mm")
                            for k in range(NKV):
                                nc.tensor.matmul(
                                    ps[:],
                                    lkv[:, k, (t - 4 * g) * P : (t - 4 * g + 1) * P],
                                    wvu_t[:, k, :],
                                    start=(k == 0),
                                    stop=(k == NKV - 1),
                                )
                            nc.vector.tensor_copy(v_sb[:, t, :], ps[:])
                        for m in range(2):
                            psA = ps2.tile([P, SC], F32, tag="mm")
                            for k in range(NKV):
                                nc.tensor.matmul(
                                    psA[:],
                                    wkr_t[:, k, m * P : (m + 1) * P],
                                    lkv[:, k, :],
                                    start=(k == 0), stop=(k == NKV - 1),
                                )
                            rope_finish(
                                kr_sb[:, m, sl], psA, bkr_sb[:, m : m + 1], sl
                            )

            # ---------- phase 3: attention + inline out-proj ----------
            with (
                tc.tile_pool(name="at", bufs=12) as pat,
                tc.tile_pool(name="atx", bufs=2) as patx,
                tc.tile_pool(name="att", bufs=2) as patt,
                tc.tile_pool(name="acc", bufs=2) as pacc,
                tc.tile_pool(name="out", bufs=3) as pout,
                tc.tile_pool(name="ow", bufs=3) as pow_,
                tc.tile_pool(name="ps_sc", bufs=3, space="PSUM") as ps_sc,
                tc.tile_pool(name="ps_acc", bufs=2, space="PSUM") as ps_acc,
                tc.tile_pool(name="ps_red", bufs=1, space="PSUM") as ps_red,
                tc.tile_pool(name="ps_m", bufs=2, space="PSUM") as ps_m,
            ):
                for qc in range(NSC):
                    nkb = 4 * qc + 4
                    ctx_q = patx.tile([P, NH, SC], BF16, tag="ctx")
                    acc = {}
                    sacc = {}
                    nstate = {}

                    def emit_scores(h, kb):
                        hc = h // 2
                        hp = (h % 2) * ROPE_DIM
                        ksl = slice(kb * P, (kb + 1) * P)
                        diag = kb >= 4 * qc
                        c = (kb - 4 * qc) * P if diag else 0
                        qs0 = qc * SC + c
                        ps = ps_sc.tile([P, SC], F32, tag="sc", name="ps")
                        nc.tensor.matmul(
                            ps[:, c:],
                            kc_sb[:, h, ksl],
                            qc_sb[:, h, qs0 : (qc + 1) * SC],
                            start=True, stop=False,
                        )
                        nc.tensor.matmul(
                            ps[:, c:],
                            kr_sb[hp : hp + ROPE_DIM, hc, ksl],
                            qr_sb[hp : hp + ROPE_DIM, hc,
                                  qs0 : (qc + 1) * SC],
                            start=False, stop=True,
                        )
                        probs = pat.tile([P, SC], BF16, tag="probs",
                                         name="probs")
                        if diag:
                            nc.vector.tensor_tensor(
                                ps[:, c : c + P],
                                ps[:, c : c + P],
                                tri_sb[:],
                                mybir.AluOpType.add,
                            )
                        nc.scalar.activation(
                            probs[:, c:], ps[:, c:], AF.Exp, scale=SCALE,
                        )
                        return (h, kb, probs, c)

                    def emit_ctx(unit):
                        h, kb, probs, c = unit
                        nc.tensor.matmul(
                            acc[h][:, c:],
                            v_sb[:, kb, h * P : (h + 1) * P],
                            probs[:, c:],
                            start=(kb == 0), stop=(kb == nkb - 1),
                        )
                        if kb == 0:
                            nc.gpsimd.tensor_copy(sacc[h][:], probs[:])
                        else:
                            nc.gpsimd.tensor_tensor(
                                sacc[h][:, c:], sacc[h][:, c:],
                                probs[:, c:], mybir.AluOpType.add,
                            )
                        return h if kb == nkb - 1 else None

                    def emit_norm_a(h):
                        acc16 = patt.tile([P, SC], BF16, tag="acc16",
                                          name="acc16")
                        nc.gpsimd.tensor_copy(acc16[:], sacc[h][:])
                        red = ps_red.tile([1, SC], F32, tag="red", name="red")
                        nc.tensor.matmul(
                            red[:], ones_col[:], acc16[:],
                            start=True, stop=True,
                        )
                        rf = patt.tile([1, SC], F32, tag="recip", name="rf")
                        nc.vector.reciprocal(rf[:], red[0:1, :])
                        r16 = patt.tile([1, SC], BF16, tag="r16", name="r16")
                        nc.vector.tensor_copy(r16[:], rf[:])
                        nstate[h] = r16

                    def emit_norm_b(h):
                        r16 = nstate.pop(h)
                        psb = ps_m.tile([P, SC], F32, tag="m", name="psb")
                        nc.tensor.matmul(
                            psb[:], ones_row[:], r16[:],
                            start=True, stop=True,
                        )
                        rbc = patt.tile([P, SC], BF16, tag="rbc", name="rbc")
                        nc.scalar.copy(rbc[:], psb[:])
                        nc.vector.tensor_tensor(
                            ctx_q[:, h, :], acc[h][:], rbc[:],
                            mybir.AluOpType.mult,
                        )

                    units = [(h, kb) for h in range(NH) for kb in range(nkb)]
                    n = len(units)
                    state = {}
                    sched = {}
                    for i in range(n + 4):
                        if i < n:
                            h, kb = units[i]
                            if kb == 0:
                                acc[h] = ps_acc.tile([P, SC], F32, tag="ctx",
                                                     name="pctx")
                                sacc[h] = pacc.tile([P, SC], F32, tag="sacc",
                                                    name="sacc")
                            state[i] = emit_scores(h, kb)
                        if 0 <= i - 1 < n:
                            h_closed = emit_ctx(state.pop(i - 1))
                            if h_closed is not None:
                                sched.setdefault(i + 1, []).append(
                                    ("a", h_closed))
                                sched.setdefault(i + 3, []).append(
                                    ("b", h_closed))
                        for kind, hh in sched.pop(i, []):
                            (emit_norm_a if kind == "a" else emit_norm_b)(hh)

                    # out-projection for this q-chunk
                    for m in range(NKT):
                        wo_t = pow_.tile([P, NH, P], BF16, tag="wo")
                        nc.sync.dma_start(
                            wo_t[:],
                            Wo[:, m * P : (m + 1) * P].rearrange(
                                "(t p) m -> p t m", p=P
                            ),
                        )
                        ps = ps_m.tile([P, SC], F32, tag="m", name="ps")
                        for k in range(NH):
                            nc.tensor.matmul(
                                ps[:],
                                wo_t[:, k, :],
                                ctx_q[:, k, :],
                                start=(k == 0),
                                stop=(k == NH - 1),
                            )
                        og = pout.tile([P, SC], BF16, tag="og")
                        nc.scalar.copy(og[:], ps[:])
                        nc.sync.dma_start(
                            outT[m * P : (m + 1) * P,
                                 qc * SC : (qc + 1) * SC],
                            og[:],
                        )
    _split_waits(nc)
    return nc


def _col_bias(b, nm):
    """[nm*128] -> [128, nm] (column m = bias for feature chunk m)."""
    return np.ascontiguousarray(b.reshape(nm, P).T).astype(np.float32)


_NC = None


def kernel(**inputs):
    global _NC
    inp = {k: np.asarray(v) for k, v in inputs.items()}
    x = inp["x"].astype(np.float32)

    pos = np.arange(S, dtype=np.float64)
    inv = 1.0 / (10000.0 ** (np.arange(0, ROPE_DIM, 2, np.float64) / ROPE_DIM))
    ang = pos[None, :] * inv[:, None]          # [32, S]
    idx = (np.arange(P) % ROPE_DIM) // 2       # row -> freq index
    cos2 = np.cos(ang)[idx].astype(BF)
    sgn = np.where(np.arange(P) % 2 == 0, -1.0, 1.0)[:, None]
    sina = (np.sin(ang)[idx] * sgn).astype(BF)
    tri = np.where(
        np.arange(P)[None, :] >= np.arange(P)[:, None], 0.0, NEG
    ).astype(np.float32)

    qdw = inp["query_down_w"].astype(np.float32)
    qdb = inp["query_down_b"].astype(np.float32)
    quw = inp["query_up_w"].astype(np.float32)
    qrw = inp["query_rope_w"].astype(np.float32)

    in_maps = []
    for c in range(8):
        b, g = c // 4, c % 4
        h0 = g * NH
        csl = slice(h0 * HEAD_DIM, (h0 + NH) * HEAD_DIM)
        rsl = slice(h0 * ROPE_DIM, (h0 + NH) * ROPE_DIM)
        wqf = qdw @ quw[:, csl]
        bqf = qdb @ quw[:, csl] + inp["query_up_b"][csl].astype(np.float32)
        wqrf = qdw @ qrw[:, rsl]
        bqrf = qdb @ qrw[:, rsl] + inp["query_rope_b"][rsl].astype(np.float32)
        xt = np.ascontiguousarray(x[b].T).astype(BF)
        in_maps.append(
            {
                "xTs": np.ascontiguousarray(xt[:, g * SC : (g + 1) * SC]),
                "xTf": xt,
                "Wd": inp["kv_down_w"].astype(BF),
                "bd": _col_bias(inp["kv_down_b"], NKV),
                "Wku": inp["key_up_w"][:, csl].astype(BF),
                "bku": _col_bias(inp["key_up_b"][csl], 4),
                "Wvu": inp["value_up_w"][:, csl].astype(BF),
                "Wkr": inp["key_rope_w"][:, rsl].astype(BF),
                "bkr": _col_bias(inp["key_rope_b"][rsl].astype(np.float32), 2),
                "Wqf": wqf.astype(BF),
                "bqf": _col_bias(bqf, 4),
                "Wqrf": wqrf.astype(BF),
                "bqrf": _col_bias(bqrf, 2),
                "Wo": inp["out_w"][csl, :].astype(BF),
                "cos2": cos2,
                "sina": sina,
                "tri": tri,
            }
        )

    if _NC is None:
        _NC = build()
    res = run_bass_kernel_spmd(_NC, in_maps, core_ids=list(range(8)))

    corr = (
        inp["value_up_b"].astype(np.float32) @ inp["out_w"].astype(np.float32)
        + inp["out_b"].astype(np.float32)
    )
    out = np.empty((B, S, HIDDEN), np.float32)
    for b in range(B):
        acc = res.results[b * 4]["outT"].astype(np.float32)
        for g in range(1, 4):
            acc += res.results[b * 4 + g]["outT"].astype(np.float32)
        out[b] = acc.T + corr[None, :]
    return out
